# revision 6
# baseline (speedup 1.0000x reference)
"""LSTM (B=131072, T=10, INP=HID=64) + linear head, data-parallel on 8 TRN2 cores.

v2: engine-rebalanced design. The baseline was ACT-bound (~400k sigmoid/tanh
columns at 0.83ns each). This version:
  - Per-gate activations with the bias folded into the ACT instruction's
    bias operand (no PE bias-seed matmuls for exact-path gates).
  - i/f gates on "approx" steps are computed by a custom DVE op SIG2MUL:
    out = (1 + v(1 + imm2*v^2))*Src1 with v = clip(z*k, +-3k) = 2*sigma~(z)*Src1,
    a minimax cubic sigmoid (max err ~0.02) fused with the gate product.
    The i/f weight blocks are pre-scaled by k = 2*c1 so the cubic's linear
    coefficient is exactly 1 (stage-budget trick: 8 ALU stages).
  - Cell state kept HALVED (c_half = c/2): SIG2MUL yields 2*sigma*Src1, so
    sigma_f * c_half arrives correctly scaled; tanh(c) runs on ACT with
    scale=2 (free un-halving); u-term uses G/4 (one 4x tensor_scalar).
  - g gate: exact tanh on ACT (cubic tanh too inaccurate); o gate: exact
    sigmoid on ACT; h products on Pool(gpsimd)/DVE.
  - Pool engine absorbs c-adds and A-half h-products via scalar_tensor_tensor.
  - Matmuls: A-half fp8 DoubleRow (fused [W_hh;W_ih], K=128) steps 1-8,
    B-half bf16, t=0 K=65 with bias row folded, t=9 bf16; PE seeds biases
    (fp8 DR K=1) only for approx-path gates on steps 1-7.
"""

import dataclasses

import numpy as np
import ml_dtypes

import concourse.bass as bass
import concourse.mybir as mybir
from concourse import bacc
import concourse.tile as tile
import concourse.dve_ops as dve_ops
from concourse.dve_ops import DveOp
from concourse.dve_uop import DveOpSpec
from concourse.dve_spec import (
    Spec, Src0, Src1, C0, C1, C2, Zero, One, maxx, minn, sq, lower, _has_src1)

HID = 64
INP = 64
T = 10
B = 131072
NCORES = 8
B_LOC = B // NCORES  # 16384
NB = 512             # batch columns per group
NUNITS = B_LOC // (2 * NB)  # 16
NPAIRS = NUNITS // 2

BF = mybir.dt.bfloat16
F32 = mybir.dt.float32
F8 = mybir.dt.float8e4
AF = mybir.ActivationFunctionType
ALU = mybir.AluOpType
PM = mybir.MatmulPerfMode

# gate order in psum/weights: 0=i, 1=f, 2=g, 3=o
GI, GF, GG, GO = 0, 1, 2, 3

# sigma~(x) = 0.5 + u(c1 + c2 u^2), u = clip(x, -a, a): minimax cubic
SA = 3.22481189
SC1 = 0.21987955
SC2 = -0.00704782
KS = 2 * SC1                      # weight pre-scale for i/f
IMM_SIG = 2 * SC2 / KS ** 3       # cubic coefficient in the scaled frame

# pairs whose i-gate stays exact on ACT during steps 1-7 (DVE is the
# steady-state bottleneck; ACT has slack). Only the i-gate may be mixed
# per-pair: per-pair f-mixing would desync the power-of-2 cell ledger.
XPAIRS_I = frozenset({1, 3, 5, 7})
# steps whose i/f gates run on the DVE cubic path (f has no gate at t=0)
APPROX_I = frozenset(range(0, 9))
APPROX_F = frozenset(range(1, 8))
# steps whose A-half gate matmuls are fp8 DoubleRow
FP8_STEPS = frozenset(range(1, 9))

_REGISTERED = {}


def _register(name, spec):
    if name in _REGISTERED:
        return _REGISTERED[name]
    op = DveOp(name, spec, subdim=False, uops_sha={})
    dve_ops.OPS.append(op)
    dve_ops._SUB_OPCODE_FOR_NAME[name] = (
        dve_ops._CUSTOM_DVE_ROW_BASE + len(dve_ops.OPS) - 1)
    shas = {}
    for ver in ("v3", "v4"):
        d = DveOpSpec(name=name, opcode=dve_ops.get_dve_sub_opcode(name),
                      uops=lower(spec, ver=ver), rd1_en=_has_src1(spec))
        shas[ver] = d.sha(ver)
    op2 = dataclasses.replace(op, uops_sha=shas)
    dve_ops.OPS[-1] = op2
    dve_ops.CUSTOM_DVE_SPECS[name] = op2.spec
    _REGISTERED[name] = op2
    return op2


def _sig2_ref(in0, in1, s0, s1, imm2):
    v = np.clip(in0.astype(np.float32), s0, s1)
    return (((np.square(v) * imm2 + 1.0) * v + 1.0) * in1).astype(np.float32)


_v = minn(maxx(Src0, C0), C1)
SIG2MUL = _register("SIG2MUL_ANT", Spec(
    body=((sq(_v) * C2 + One) * _v + One) * Src1, reference=_sig2_ref))


def emit_lstm(tc, aps, units=NUNITS, steps=T):
    nc = tc.nc
    xt = aps["xt"]
    npairs = units // 2

    with (
        tc.tile_pool(name="const", bufs=1) as cpool,
        tc.tile_pool(name="xbuf", bufs=2) as xpool,
        tc.tile_pool(name="state", bufs=2) as spool,
        tc.tile_pool(name="work", bufs=3) as wpool,
        tc.tile_pool(name="hout", bufs=units) as hpool,
        tc.tile_pool(name="ps1", bufs=1, space="PSUM") as ppool1,
    ):
        # ---- startup DMAs ------------------------------------------------
        W0_sb = cpool.tile([65, 512], BF)
        nc.gpsimd.dma_start(out=W0_sb, in_=aps["W0d"])
        W8_sb = cpool.tile([64, 2, 256], F8)
        Wb_sb = cpool.tile([128, 256], BF)
        W9_sb = cpool.tile([128, 512], BF)
        BW8_sb = cpool.tile([1, 2, 4, 128], F8)
        Bact_sb = cpool.tile([128, 4], F32)
        WO_sb = cpool.tile([128, 2], BF)
        BO_sb = cpool.tile([2, 1], F32)
        ones8_sb = cpool.tile([1, 2, NB], F8)
        nc.vector.memset(ones8_sb, 1.0)

        xr = xt.rearrange("t p (u g n) -> t p u g n", g=2, n=NB)
        f8steps = sorted(FP8_STEPS)
        xr8 = aps["xt8"].rearrange("t p (u g n) -> t p u g n", g=2, n=NB)

        A2 = [None] * steps
        B2 = [None] * steps

        def load_x(t, chunk=units):
            # B-half rhs: [x_B(0:64); h_B(64:128)] bf16, all steps
            b = xpool.tile([128, units, NB], BF, tag="B2", name=f"B2_{t}")
            if t in FP8_STEPS:
                # A-half rhs for DR: [64, k2, units, NB] fp8; k2=0 h, k2=1 x
                a = xpool.tile([64, 2, units, NB], F8, tag="A2", name=f"A8_{t}")
                ti = f8steps.index(t)
                nc.sync.dma_start(out=a[:, 1, :, :], in_=xr8[ti, :, :, 0, :])
            else:
                # bf16 A-half rhs: t=0: [x(0:64); ones(64)]; t=9: [h(0:64); x(64:128)]
                a = xpool.tile([128, units, NB], BF, tag="A2", name=f"A2_{t}")
                arows = slice(0, 64) if t == 0 else slice(64, 128)
                for u0 in range(0, units, chunk):
                    u1 = u0 + chunk
                    nc.sync.dma_start(out=a[arows, u0:u1, :],
                                      in_=xr[t, :, u0:u1, 0, :])
            for u0 in range(0, units, chunk):
                u1 = u0 + chunk
                nc.sync.dma_start(out=b[0:64, u0:u1, :],
                                  in_=xr[t, :, u0:u1, 1, :])
            A2[t], B2[t] = a, b

        # t=0 ones rows first (tiny), then first x chunks
        A2[0] = xpool.tile([128, units, NB], BF, tag="A2", name="A2_0")
        B2[0] = xpool.tile([128, units, NB], BF, tag="B2", name="B2_0")
        e1 = aps["E1d"].rearrange("q (u n) -> q u n", n=NB)
        nc.gpsimd.dma_start(out=A2[0][64:65, :, :], in_=e1)
        nc.gpsimd.dma_start(out=B2[0][64:65, :, :], in_=e1)
        nc.gpsimd.dma_start(out=W8_sb, in_=aps["W8d"])
        nc.gpsimd.dma_start(out=Wb_sb, in_=aps["Wbd"])
        nc.gpsimd.dma_start(out=BW8_sb, in_=aps["BW8d"])
        nc.gpsimd.dma_start(out=Bact_sb, in_=aps["Bactd"])
        nc.gpsimd.dma_start(out=W9_sb, in_=aps["W9d"])
        nc.gpsimd.dma_start(out=WO_sb, in_=aps["WOd"])
        nc.gpsimd.dma_start(out=BO_sb, in_=aps["BOd"])
        for u0 in range(0, units, 1):
            u1 = u0 + 1
            nc.sync.dma_start(out=A2[0][0:64, u0:u1, :], in_=xr[0, :, u0:u1, 0, :])
            nc.sync.dma_start(out=B2[0][0:64, u0:u1, :], in_=xr[0, :, u0:u1, 1, :])
        load_x(1)

        # stored cell state y_t = 2^{e_t} * c_t: every f-approx step doubles
        # the state (SIG2MUL yields 2*sigma*Src1); the ledger exponent is
        # absorbed into the Gq scale and the tanh input scale.
        eled = []
        e = 0
        for t in range(steps):
            if t > 0 and t in APPROX_F:
                e += 1
            eled.append(e)

        C = [None] * npairs    # ledgered cell state, [128, 2, NB] per pair
        Hf = [None] * units    # final h tiles for the head
        yb = cpool.tile([2, units, NB], F32, tag="yb", name="yb")

        def head(q):
            # output head for a quad of units; psum rides the zg/zo rings
            op4 = ppool1.tile([2, 2, NB], F32, tag="zg" if q % 2 == 0 else "zo",
                              name=f"op4_{q}")
            for k in range(2):
                nc.tensor.matmul(op4[:, k, :], WO_sb, Hf[2 * q + k],
                                 start=True, stop=True, skip_group_check=True)
            if q % 2 == 1:
                nc.scalar.activation(yb[:, 2 * q:2 * q + 2, :], op4,
                                     AF.Identity, bias=BO_sb)
            else:
                nc.vector.tensor_scalar_add(yb[:, 2 * q:2 * q + 2, :], op4,
                                            BO_sb)

        def fl(ap):
            # [128, 2, NB] -> [128, 1024] single-free-dim view
            return ap.rearrange("p a b -> p (a b)")

        # pending (TH, h) work at 1-pair lag so ACT/DVE queues don't
        # head-block on the c-chain
        pend = []

        def post_th(item):
            j, O, Cn, t = item
            TH = wpool.tile([128, 2, NB], BF, tag="TH", name=f"th_{t}_{j}")
            last = t == steps - 1
            thsc = float(2.0 ** (-eled[t]))
            if last and j == npairs - 1:
                nc.scalar.activation(TH[:, 0, :], Cn[:, 0, :], AF.Tanh, scale=thsc)
                nc.scalar.activation(TH[:, 1, :], Cn[:, 1, :], AF.Tanh, scale=thsc)
            else:
                nc.scalar.activation(fl(TH), fl(Cn), AF.Tanh, scale=thsc)
            return TH

        def post_h(item, TH):
            j, O, Cn, t = item
            last = t == steps - 1
            if last:
                for uh in range(2):
                    u = 2 * j + uh
                    Hf[u] = hpool.tile([128, NB], BF, tag="Hf", name=f"hf_{u}")
                    nc.vector.tensor_mul(Hf[u], O[:, uh, :], TH[:, uh, :])
                return
            tn = t + 1
            if tn in FP8_STEPS:
                ha_dst = A2[tn][0:64, 0, 2 * j:2 * j + 2, :]
            else:
                ha_dst = A2[tn][0:64, 2 * j:2 * j + 2, :]
            # A-half h product on Pool (fp8 output would be 1x on DVE anyway)
            nc.gpsimd.tensor_mul(ha_dst, O[0:64, :, :], TH[0:64, :, :])
            hb_eng = nc.gpsimd if m % 2 == 1 else nc.vector
            hb_eng.tensor_mul(B2[tn][64:128, 2 * j:2 * j + 2, :],
                              O[64:128, :, :], TH[64:128, :, :])

        for t in range(steps):
            if t + 2 < steps:
                load_x(t + 2)
            for j in range(npairs):
                ai = t in APPROX_I and not (1 <= t <= 7 and j in XPAIRS_I)
                af = t in APPROX_F
                zg = ppool1.tile([128, 2, NB], F32, tag="zg", name=f"zg_{t}_{j}")
                zo = ppool1.tile([128, 2, NB], F32, tag="zo", name=f"zo_{t}_{j}")
                zi = ppool1.tile([128, 2, NB], F32, tag="zi", name=f"zi_{t}_{j}")
                zf = None
                if t > 0:
                    zf = ppool1.tile([128, 2, NB], F32, tag="zf",
                                     name=f"zf_{t}_{j}")
                for uh in range(2):
                    u = 2 * j + uh

                    def mm(dst, s, seed):
                        if t == 0:
                            co = s * 128
                            nc.tensor.matmul(
                                dst[0:64, :], W0_sb[:, co:co + 64],
                                A2[0][0:65, u, :], start=True, stop=True,
                                skip_group_check=True)
                            nc.tensor.matmul(
                                dst[64:128, :], W0_sb[:, co + 64:co + 128],
                                B2[0][0:65, u, :], start=True, stop=True,
                                skip_group_check=True)
                            return
                        if t in FP8_STEPS:
                            if seed:
                                nc.tensor.matmul(dst, BW8_sb[:, :, s, :],
                                                 ones8_sb, start=True,
                                                 stop=False,
                                                 perf_mode=PM.DoubleRow,
                                                 skip_group_check=True)
                            c8 = s * 64
                            # without a full-width seed each half must open
                            # its own accumulation group (start=True)
                            nc.tensor.matmul(
                                dst[0:64, :], W8_sb[:, :, c8:c8 + 64],
                                A2[t][:, :, u, :], start=not seed,
                                stop=not seed,
                                perf_mode=PM.DoubleRow, skip_group_check=True)
                            nc.tensor.matmul(
                                dst[64:128, :], Wb_sb[:, c8:c8 + 64],
                                B2[t][:, u, :], start=not seed, stop=True,
                                skip_group_check=True)
                        else:  # t == 9, bf16 both halves
                            co = s * 128
                            nc.tensor.matmul(
                                dst[0:64, :], W9_sb[:, co:co + 64],
                                A2[t][:, u, :], start=True, stop=True,
                                skip_group_check=True)
                            nc.tensor.matmul(
                                dst[64:128, :], W9_sb[:, co + 64:co + 128],
                                B2[t][:, u, :], start=True, stop=True,
                                skip_group_check=True)

                    mm(zg[:, uh, :], GG, seed=False)
                    mm(zo[:, uh, :], GO, seed=False)
                    mm(zi[:, uh, :], GI, seed=ai and t > 0)
                    if t > 0:
                        mm(zf[:, uh, :], GF, seed=af)

                # ---- consumers for pair j --------------------------------
                # t=0: biases already folded into W0's ones row
                bias_of = (lambda s: Bact_sb[:, s:s + 1]) if t > 0 else (
                    lambda s: 0.0)
                G = wpool.tile([128, 2, NB], BF, tag="G", name=f"g_{t}_{j}")
                nc.scalar.activation(fl(G), fl(zg), AF.Tanh, bias=bias_of(GG))
                O = wpool.tile([128, 2, NB], BF, tag="O", name=f"o_{t}_{j}")
                nc.scalar.activation(fl(O), fl(zo), AF.Sigmoid,
                                     bias=bias_of(GO))
                Gq = wpool.tile([128, 2, NB], BF, tag="Gq", name=f"gq_{t}_{j}")
                gq_scale = float(2.0 ** (eled[t] - 1)) if ai else float(
                    2.0 ** eled[t])
                nc.vector.tensor_scalar(fl(Gq), fl(G), gq_scale, 0.0,
                                        ALU.mult, ALU.add)
                Cn = spool.tile([128, 2, NB], BF, tag=f"C{j}", name=f"c_{t}_{j}")
                # at t=0, c0_half = u_half: the u ops write straight into Cn
                U = Cn if t == 0 else wpool.tile([128, 2, NB], BF, tag="U",
                                                 name=f"u_{t}_{j}")
                if ai:
                    nc.vector._custom_dve(
                        SIG2MUL, out=fl(U), in0=fl(zi), in1=fl(Gq),
                        s0=-SA * KS, s1=SA * KS, imm2=IMM_SIG)
                else:
                    I = wpool.tile([128, 2, NB], BF, tag="I", name=f"i_{t}_{j}")
                    nc.scalar.activation(fl(I), fl(zi), AF.Sigmoid,
                                         bias=bias_of(GI), scale=1.0 / KS)
                    nc.vector.tensor_mul(fl(U), fl(I), fl(Gq))
                if t > 0:
                    V = wpool.tile([128, 2, NB], BF, tag="V", name=f"v_{t}_{j}")
                    if af:
                        nc.vector._custom_dve(
                            SIG2MUL, out=fl(V), in0=fl(zf), in1=fl(C[j]),
                            s0=-SA * KS, s1=SA * KS, imm2=IMM_SIG)
                    else:
                        F = wpool.tile([128, 2, NB], BF, tag="F",
                                       name=f"f_{t}_{j}")
                        nc.scalar.activation(fl(F), fl(zf), AF.Sigmoid,
                                             bias=bias_of(GF), scale=1.0 / KS)
                        nc.vector.tensor_mul(fl(V), fl(F), fl(C[j]))
                    # c' = u + v on DVE (keeps the c-chain off the slow Pool)
                    nc.vector.tensor_add(fl(Cn), fl(U), fl(V))
                C[j] = Cn

                if pend and pend[-1][1] is None:
                    pend[-1][1] = post_th(pend[-1][0])
                if len(pend) >= 2:
                    it, th = pend.pop(0)
                    post_h(it, th)
                pend.append([(j, O, Cn, t), None])
                if t == steps - 1 and j in (6, 7):
                    head(2 * (j - 6))
                    head(2 * (j - 6) + 1)
            # drain pending pairs of the step
            for it in pend:
                if it[1] is None:
                    it[1] = post_th(it[0])
            for it, th in pend:
                post_h(it, th)
            pend = []

        yr = aps["y"].rearrange("(u p n) -> p u n", p=2, n=NB)
        head(4)
        head(5)
        nc.sync.dma_start(out=yr[:, 0:8, :], in_=yb[:, 0:8, :])
        head(6)
        head(7)
        nc.sync.dma_start(out=yr[:, 8:16, :], in_=yb[:, 8:16, :])


def prep_weights(W_ih, W_hh, b_ih, b_hh, W_out, b_out):
    bf16 = ml_dtypes.bfloat16
    fp8 = ml_dtypes.float8_e4m3
    b = (b_ih + b_hh).astype(np.float32)
    scale = np.array([KS, KS, 1.0, 1.0], np.float32)

    W8 = np.zeros((64, 2, 256), np.float32)
    Wb = np.zeros((128, 256), np.float32)
    W9 = np.zeros((128, 512), np.float32)
    W0 = np.zeros((65, 512), np.float32)
    BW8v = np.zeros((4, 128), np.float32)
    Bact = np.zeros((128, 4), np.float32)
    for s in range(4):
        blk_ih = W_ih[s * 64:(s + 1) * 64, :].astype(np.float32)
        blk_hh = W_hh[s * 64:(s + 1) * 64, :].astype(np.float32)
        ss = scale[s]
        co64 = s * 64
        co = s * 128
        # A-half DR lhsT: partition p holds (h_p, x_p) k-pair
        W8[:, 0, co64:co64 + 64] = blk_hh.T * ss
        W8[:, 1, co64:co64 + 64] = blk_ih.T * ss
        # B-half bf16 lhsT: rows [x(0:64); h(64:128)]
        Wb[0:64, co64:co64 + 64] = blk_ih.T * ss
        Wb[64:128, co64:co64 + 64] = blk_hh.T * ss
        # t=9 bf16: A cols rows [h; x], B cols rows [x; h]
        W9[0:64, co:co + 64] = blk_hh.T * ss
        W9[64:128, co:co + 64] = blk_ih.T * ss
        W9[0:64, co + 64:co + 128] = blk_ih.T * ss
        W9[64:128, co + 64:co + 128] = blk_hh.T * ss
        # t=0 K=65 with bias row (h == 0)
        bb = b[s * 64:(s + 1) * 64] * ss
        W0[0:64, co:co + 64] = blk_ih.T * ss
        W0[64, co:co + 64] = bb
        W0[0:64, co + 64:co + 128] = blk_ih.T * ss
        W0[64, co + 64:co + 128] = bb
        BW8v[s, 0:64] = bb
        BW8v[s, 64:128] = bb
        Bact[0:64, s] = b[s * 64:(s + 1) * 64]
        Bact[64:128, s] = b[s * 64:(s + 1) * 64]
    # DoubleRow seeds: two fp8 rows summing to the bf16-exact scaled bias
    b1 = BW8v.astype(fp8).astype(np.float32)
    b2 = (BW8v - b1).astype(fp8).astype(np.float32)
    BW8 = np.zeros((1, 2, 4, 128), np.float32)
    BW8[0, 0] = b1
    BW8[0, 1] = b2
    WO = np.zeros((128, 2), np.float32)
    WO[0:64, 0] = W_out[0].astype(np.float32)
    WO[64:128, 1] = W_out[0].astype(np.float32)
    BO = np.full((2, 1), np.float32(b_out[0]))
    return {
        "W8d": W8.astype(fp8),
        "Wbd": Wb.astype(bf16),
        "W9d": W9.astype(bf16),
        "W0d": W0.astype(bf16),
        "BW8d": BW8.astype(fp8),
        "Bactd": Bact.astype(np.float32),
        "E1d": np.ones((1, B_LOC // 2), np.float32).astype(bf16),
        "WOd": WO.astype(bf16),
        "BOd": BO,
    }


_BUILD_CACHE = {}


def build_nc():
    key = ("nc2",)
    if key in _BUILD_CACHE:
        return _BUILD_CACHE[key]
    nc = bacc.Bacc("TRN2", target_bir_lowering=False, debug=False)
    aps = {
        "xt": nc.dram_tensor("xt", [T, INP, B_LOC], BF, kind="ExternalInput").ap(),
        "xt8": nc.dram_tensor("xt8", [len(FP8_STEPS), INP, B_LOC], F8,
                              kind="ExternalInput").ap(),
        "W8d": nc.dram_tensor("W8d", [64, 2, 256], F8, kind="ExternalInput").ap(),
        "Wbd": nc.dram_tensor("Wbd", [128, 256], BF, kind="ExternalInput").ap(),
        "W9d": nc.dram_tensor("W9d", [128, 512], BF, kind="ExternalInput").ap(),
        "W0d": nc.dram_tensor("W0d", [65, 512], BF, kind="ExternalInput").ap(),
        "BW8d": nc.dram_tensor("BW8d", [1, 2, 4, 128], F8,
                               kind="ExternalInput").ap(),
        "Bactd": nc.dram_tensor("Bactd", [128, 4], F32,
                                kind="ExternalInput").ap(),
        "E1d": nc.dram_tensor("E1d", [1, B_LOC // 2], BF,
                              kind="ExternalInput").ap(),
        "WOd": nc.dram_tensor("WOd", [128, 2], BF, kind="ExternalInput").ap(),
        "BOd": nc.dram_tensor("BOd", [2, 1], F32, kind="ExternalInput").ap(),
        "y": nc.dram_tensor("y", [B_LOC], F32, kind="ExternalOutput").ap(),
    }
    with tile.TileContext(nc) as tc:
        emit_lstm(tc, aps)
    nc.compile()
    _BUILD_CACHE[key] = nc
    return nc


def make_in_maps(x, W_ih, W_hh, b_ih, b_hh, W_out, b_out):
    bf16 = ml_dtypes.bfloat16
    wd = prep_weights(W_ih, W_hh, b_ih, b_hh, W_out, b_out)
    xtf = np.ascontiguousarray(x.transpose(1, 2, 0))  # [T, I, B]
    xt = xtf.astype(bf16)
    f8steps = sorted(FP8_STEPS)
    xt8 = np.ascontiguousarray(xtf[f8steps]).astype(ml_dtypes.float8_e4m3)
    in_maps = []
    for c in range(NCORES):
        sl = np.ascontiguousarray(xt[:, :, c * B_LOC:(c + 1) * B_LOC])
        sl8 = np.ascontiguousarray(xt8[:, :, c * B_LOC:(c + 1) * B_LOC])
        in_maps.append({"xt": sl, "xt8": sl8, **wd})
    return in_maps


def kernel(x, W_ih, W_hh, b_ih, b_hh, W_out, b_out):
    from concourse.bass_utils import run_bass_kernel_spmd

    nc = build_nc()
    in_maps = make_in_maps(x, W_ih, W_hh, b_ih, b_hh, W_out, b_out)
    res = run_bass_kernel_spmd(nc, in_maps, core_ids=list(range(NCORES)))
    y = np.concatenate([res.results[c]["y"] for c in range(NCORES)])
    return y.reshape(B, 1).astype(np.float32)
